# revision 2
# baseline (speedup 1.0000x reference)
"""CrossModalMatchingNetwork Trainium2 kernel.

Full-input contract: kernel(**inputs) takes the unsharded numpy inputs and
returns the full [B, S, S] cosine-similarity output.

Strategy: data-parallel over batch across 8 NeuronCores (2 batches/core).
Host-side prep transposes the big activations to [D, S] layout so the
contraction dim lands on SBUF partitions, and replicates the small
projection weights (pre-transposed to [D, H]) to every core.

Per core, per batch:
  vT[h,s]  = sum_d WvT[d,h] * visT[d,s] + bv[h]     (fp32r matmuls, fp32 acc)
  tT[h,s]  = sum_d WtT[d,h] * txtT[d,s] + bt[h]
  vn2[s]   = sum_h vT[h,s]^2   (DVE squares + ones-vector matmul)
  tn2[s]   = sum_h tT[h,s]^2
  vT      *= 1/sqrt(vn2)       (row replicated via K=1 matmul, DVE mul)
  dots     = vT.T @ tT         (fp32r matmuls)
  out      = dots * 1/sqrt(tn2)  (DVE epilogue)
"""

import numpy as np
from contextlib import ExitStack

import concourse.bass as bass
import concourse.mybir as mybir
import concourse.tile as tile
from concourse import bacc
from concourse.bass import ds, ts

B, S, VD, TD, H = 16, 1024, 1024, 768, 512
NCORES = 8
BPC = B // NCORES  # batches per core
P = 128
FD = 512  # matmul moving-operand free dim (fp32 max; also one PSUM bank)

F32 = mybir.dt.float32
F32R = mybir.dt.float32r

AF = mybir.ActivationFunctionType


def _r(ap):
    """Reinterpret an fp32 AP as fp32r so the PE runs single-pass (FP22)."""
    return ap.bitcast(F32R)


def build(bpc=BPC, s=S, vd=VD, td=TD, h=H):
    fd = min(FD, s)
    kv, kt, mh = vd // P, td // P, h // P
    ns, ms = s // fd, s // P

    nc = bacc.Bacc("TRN2", target_bir_lowering=False)
    visT = nc.dram_tensor("visT", [bpc, vd, s], F32, kind="ExternalInput")
    txtT = nc.dram_tensor("txtT", [bpc, td, s], F32, kind="ExternalInput")
    wvT = nc.dram_tensor("wvT", [vd, h], F32, kind="ExternalInput")
    wtT = nc.dram_tensor("wtT", [td, h], F32, kind="ExternalInput")
    bvp = nc.dram_tensor("bvp", [P, mh], F32, kind="ExternalInput")
    btp = nc.dram_tensor("btp", [P, mh], F32, kind="ExternalInput")
    onesd = nc.dram_tensor("ones", [P, P], F32, kind="ExternalInput")
    out = nc.dram_tensor("out", [bpc, s, s], F32, kind="ExternalOutput")

    with (
        tile.TileContext(nc) as tc,
        ExitStack() as ctx,
        nc.allow_low_precision(reason="fp32r inputs are fp22-rounded by design"),
    ):
        consts = ctx.enter_context(tc.tile_pool(name="consts", bufs=1))
        vis_pool = ctx.enter_context(tc.tile_pool(name="vis", bufs=1))
        txt_pool = ctx.enter_context(tc.tile_pool(name="txt", bufs=1))
        vt_pool = ctx.enter_context(tc.tile_pool(name="vt", bufs=1))
        tt_pool = ctx.enter_context(tc.tile_pool(name="tt", bufs=1))
        sq_pool = ctx.enter_context(tc.tile_pool(name="sq", bufs=1))
        row_pool = ctx.enter_context(tc.tile_pool(name="rows", bufs=2))
        rtn_pool = ctx.enter_context(tc.tile_pool(name="rtn", bufs=2))
        out_pool = ctx.enter_context(tc.tile_pool(name="outs", bufs=2))
        ps_mm = ctx.enter_context(tc.tile_pool(name="ps_mm", bufs=3, space="PSUM"))
        ps_repl = ctx.enter_context(tc.tile_pool(name="ps_repl", bufs=2, space="PSUM"))
        ps_norm = ctx.enter_context(tc.tile_pool(name="ps_norm", bufs=2, space="PSUM"))

        # --- constants: weights (as [P, k, h]), partition-major biases, ones
        wv_sb = consts.tile([P, kv, h], F32)
        for k in range(kv):
            nc.sync.dma_start(_r(wv_sb[:, k, :]), _r(wvT[ds(k * P, P), :]))
        wt_sb = consts.tile([P, kt, h], F32)
        for k in range(kt):
            nc.sync.dma_start(_r(wt_sb[:, k, :]), _r(wtT[ds(k * P, P), :]))
        bv_sb = consts.tile([P, mh], F32)
        nc.sync.dma_start(bv_sb[:], bvp[:, :])
        bt_sb = consts.tile([P, mh], F32)
        nc.sync.dma_start(bt_sb[:], btp[:, :])
        ones_sb = consts.tile([P, P], F32)
        nc.sync.dma_start(_r(ones_sb[:]), _r(onesd[:, :]))
        ones_col = ones_sb[:, 0:1]
        ones_row = ones_sb[0:1, :]

        def proj(m_range, kk, w_sb, b_sb, x_sb, y_sb, ysq_sb):
            """y[:, m, :] = W[:, :, m-slice].T @ x + b ; ysq = y*y"""
            for m in m_range:
                for n2 in range(ns):
                    pv = ps_mm.tile([P, fd], F32, tag="ps_mm")
                    for k in range(kk):
                        nc.tensor.matmul(
                            pv[:],
                            _r(w_sb[:, k, ts(m, P)]),
                            _r(x_sb[:, k, ds(n2 * fd, fd)]),
                            start=(k == 0),
                            stop=(k == kk - 1),
                        )
                    nc.scalar.activation(
                        _r(y_sb[:, m, ds(n2 * fd, fd)]), pv[:], AF.Identity,
                        bias=b_sb[:, ds(m, 1)],
                    )
                    nc.vector.tensor_mul(
                        _r(ysq_sb[:, m, ds(n2 * fd, fd)]),
                        y_sb[:, m, ds(n2 * fd, fd)],
                        y_sb[:, m, ds(n2 * fd, fd)],
                    )

        def norm_rows(ysq_sb, tag):
            """Per-column 1/sqrt(sum_h ysq) as ns rows of [1, fd]."""
            rrows = []
            for n2 in range(ns):
                pn = ps_norm.tile([1, fd], F32, tag="ps_norm")
                for m in range(mh):
                    nc.tensor.matmul(
                        pn[:],
                        _r(ones_col),
                        _r(ysq_sb[:, m, ds(n2 * fd, fd)]),
                        start=(m == 0),
                        stop=(m == mh - 1),
                    )
                nrow = row_pool.tile([1, fd], F32, tag=f"n_{tag}{n2}")
                nc.scalar.activation(nrow[:], pn[:], AF.Sqrt)
                rrow = row_pool.tile([1, fd], F32, tag=f"r_{tag}{n2}")
                nc.vector.reciprocal(_r(rrow[:]), nrow[:])
                rrows.append(rrow)
            return rrows

        for b in range(bpc):
            # --- input loads (per-k chunks; Tile orders/overlaps them)
            vis_sb = vis_pool.tile([P, kv, s], F32)
            for k in range(kv):
                nc.sync.dma_start(_r(vis_sb[:, k, :]), _r(visT[b, ds(k * P, P), :]))
            txt_sb = txt_pool.tile([P, kt, s], F32)
            for k in range(kt):
                nc.sync.dma_start(_r(txt_sb[:, k, :]), _r(txtT[b, ds(k * P, P), :]))

            vt_sb = vt_pool.tile([P, mh, s], F32)
            tt_sb = tt_pool.tile([P, mh, s], F32)
            vsq_sb = sq_pool.tile([P, mh, s], F32, tag="vsq")
            tsq_sb = sq_pool.tile([P, mh, s], F32, tag="tsq")

            # --- projections + v-norm chain interleaved to keep PE dense
            proj(range(mh), kv, wv_sb, bv_sb, vis_sb, vt_sb, vsq_sb)
            proj(range(0, mh // 2), kt, wt_sb, bt_sb, txt_sb, tt_sb, tsq_sb)
            rvn_rows = norm_rows(vsq_sb, "v")
            proj(range(mh // 2, mh), kt, wt_sb, bt_sb, txt_sb, tt_sb, tsq_sb)

            # --- replicate 1/vn across partitions, fold into vT
            rvn_ps = []
            for n2 in range(ns):
                rp = ps_repl.tile([P, fd], F32, tag="ps_repl")
                nc.tensor.matmul(rp[:], _r(ones_row), _r(rvn_rows[n2][:]))
                rvn_ps.append(rp)
            for m in range(mh):
                for n2 in range(ns):
                    nc.vector.tensor_mul(
                        _r(vt_sb[:, m, ds(n2 * fd, fd)]),
                        vt_sb[:, m, ds(n2 * fd, fd)],
                        rvn_ps[n2][:],
                    )

            rtn_rows = norm_rows(tsq_sb, "t")
            rtn_bc = rtn_pool.tile([P, s], F32)

            # --- dots + epilogue
            out_sb = None
            for i in range(ms):
                pds = []
                for jc in range(ns):
                    pd = ps_mm.tile([P, fd], F32, tag="ps_mm")
                    for hc in range(mh):
                        nc.tensor.matmul(
                            pd[:],
                            _r(vt_sb[:, hc, ts(i, P)]),
                            _r(tt_sb[:, hc, ds(jc * fd, fd)]),
                            start=(hc == 0),
                            stop=(hc == mh - 1),
                        )
                    pds.append(pd)
                if i == 0:
                    # replicate 1/tn (emitted after dots i=0 so PE stays busy
                    # while the t-norm row chain completes)
                    for jc in range(ns):
                        rp = ps_repl.tile([P, fd], F32, tag="ps_repl")
                        nc.tensor.matmul(rp[:], _r(ones_row), _r(rtn_rows[jc][:]))
                        nc.scalar.activation(rtn_bc[:, ds(jc * fd, fd)], rp[:], AF.Copy)
                if i % 2 == 0:
                    out_sb = out_pool.tile([P, 2, s], F32)
                for jc in range(ns):
                    nc.vector.tensor_mul(
                        out_sb[:, i % 2, ds(jc * fd, fd)],
                        pds[jc][:],
                        rtn_bc[:, ds(jc * fd, fd)],
                    )
                if i % 2 == 1:
                    nc.sync.dma_start(
                        out[b, ds((i - 1) * P, 2 * P), :].rearrange(
                            "(t p) s -> p t s", p=P
                        ),
                        out_sb[:],
                    )

    nc.compile()
    return nc


_ONES = np.ones((P, P), dtype=np.float32)

_CACHE = {}


def _get_nc():
    if "nc" not in _CACHE:
        _CACHE["nc"] = build()
    return _CACHE["nc"]


def _prep_in_maps(visual_features, text_features, Wv, bv, Wt, bt):
    f = np.float32
    wvT = np.ascontiguousarray(np.asarray(Wv, dtype=f).T)  # [VD, H]
    wtT = np.ascontiguousarray(np.asarray(Wt, dtype=f).T)  # [TD, H]
    bvp = np.ascontiguousarray(np.asarray(bv, dtype=f).reshape(H // P, P).T)
    btp = np.ascontiguousarray(np.asarray(bt, dtype=f).reshape(H // P, P).T)
    vis = np.asarray(visual_features, dtype=f)
    txt = np.asarray(text_features, dtype=f)
    in_maps = []
    for c in range(NCORES):
        sl = slice(c * BPC, (c + 1) * BPC)
        in_maps.append({
            "visT": np.ascontiguousarray(vis[sl].transpose(0, 2, 1)),
            "txtT": np.ascontiguousarray(txt[sl].transpose(0, 2, 1)),
            "wvT": wvT,
            "wtT": wtT,
            "bvp": bvp,
            "btp": btp,
            "ones": _ONES,
        })
    return in_maps


def run(inputs, trace=False, tmpdir=None):
    """Returns (full_output, BassKernelResults)."""
    from concourse.bass_utils import run_bass_kernel_spmd

    nc = _get_nc()
    in_maps = _prep_in_maps(**inputs)
    res = run_bass_kernel_spmd(
        nc, in_maps, core_ids=list(range(NCORES)), trace=trace, tmpdir=tmpdir
    )
    outp = np.concatenate([res.results[c]["out"] for c in range(NCORES)], axis=0)
    return outp, res


def kernel(**inputs) -> np.ndarray:
    outp, _ = run(inputs, trace=False)
    return outp


# revision 3
# speedup vs baseline: 1.0141x; 1.0141x over previous
"""CrossModalMatchingNetwork Trainium2 kernel.

Full-input contract: kernel(**inputs) takes the unsharded numpy inputs and
returns the full [B, S, S] cosine-similarity output (float32).

Strategy: data-parallel over batch across 8 NeuronCores (2 batches/core).
Host-side prep transposes the big activations to [D, S] layout so the
contraction dim lands on SBUF partitions, casts them to bf16 (fp32 PSUM
accumulation keeps the error ~3e-3), and replicates the small projection
weights (pre-transposed to [D, H]) to every core.

Per core, per batch:
  vT[h,s]  = sum_d WvT[d,h] * visT[d,s] + bv[h]     (bf16 matmuls, fp32 acc)
  tT[h,s]  = sum_d WtT[d,h] * txtT[d,s] + bt[h]
  vn2[s]   = sum_h vT[h,s]^2   (DVE squares + ones-vector matmul)
  tn2[s]   = sum_h tT[h,s]^2
  vT      *= 1/sqrt(vn2)       (sqrt row -> replicate via K=1 matmul -> DVE
                                reciprocal on all 128 lanes -> fold into vT)
  dots     = vT.T @ tT
  out      = dots * 1/sqrt(tn2)  (DVE epilogue, fp32 out)
"""

import numpy as np
from contextlib import ExitStack

import concourse.bass as bass
import concourse.mybir as mybir
import concourse.tile as tile
from concourse import bacc
from concourse.bass import ds, ts

B, S, VD, TD, H = 16, 1024, 1024, 768, 512
NCORES = 8
BPC = B // NCORES  # batches per core
P = 128
FD = 512  # matmul moving-operand free dim (one PSUM bank of fp32)

F32 = mybir.dt.float32
F32R = mybir.dt.float32r
BF16 = mybir.dt.bfloat16

AF = mybir.ActivationFunctionType


def build(bpc=BPC, s=S, vd=VD, td=TD, h=H, dtype="bf16"):
    fd = min(FD, s)
    kv, kt, mh = vd // P, td // P, h // P
    ns, ms = s // fd, s // P

    if dtype == "bf16":
        CT = BF16
        _w = lambda ap: ap  # noqa: E731
    else:
        CT = F32
        _w = lambda ap: ap.bitcast(F32R)  # noqa: E731  # fp32r: single-pass PE

    nc = bacc.Bacc("TRN2", target_bir_lowering=False)
    visT = nc.dram_tensor("visT", [bpc, vd, s], CT, kind="ExternalInput")
    txtT = nc.dram_tensor("txtT", [bpc, td, s], CT, kind="ExternalInput")
    wvT = nc.dram_tensor("wvT", [vd, h], CT, kind="ExternalInput")
    wtT = nc.dram_tensor("wtT", [td, h], CT, kind="ExternalInput")
    bvp = nc.dram_tensor("bvp", [P, mh], F32, kind="ExternalInput")
    btp = nc.dram_tensor("btp", [P, mh], F32, kind="ExternalInput")
    onesd = nc.dram_tensor("ones", [P, P], CT, kind="ExternalInput")
    out = nc.dram_tensor("out", [bpc, s, s], F32, kind="ExternalOutput")

    with (
        tile.TileContext(nc) as tc,
        ExitStack() as ctx,
        nc.allow_low_precision(reason="compute dtype is bf16/fp32r by design"),
    ):
        consts = ctx.enter_context(tc.tile_pool(name="consts", bufs=1))
        vis_pool = ctx.enter_context(tc.tile_pool(name="vis", bufs=1))
        txt_pool = ctx.enter_context(tc.tile_pool(name="txt", bufs=1))
        vt_pool = ctx.enter_context(tc.tile_pool(name="vt", bufs=1))
        tt_pool = ctx.enter_context(tc.tile_pool(name="tt", bufs=1))
        sq_pool = ctx.enter_context(tc.tile_pool(name="sq", bufs=1))
        row_pool = ctx.enter_context(tc.tile_pool(name="rows", bufs=2))
        rvn_pool = ctx.enter_context(tc.tile_pool(name="rvn", bufs=2))
        rtn_pool = ctx.enter_context(tc.tile_pool(name="rtn", bufs=2))
        out_pool = ctx.enter_context(tc.tile_pool(name="outs", bufs=2))
        ps_mm = ctx.enter_context(tc.tile_pool(name="ps_mm", bufs=3, space="PSUM"))
        ps_repl = ctx.enter_context(tc.tile_pool(name="ps_repl", bufs=2, space="PSUM"))
        ps_norm = ctx.enter_context(tc.tile_pool(name="ps_norm", bufs=2, space="PSUM"))

        # --- constants: weights (as [P, k, h]), partition-major biases, ones
        wv_sb = consts.tile([P, kv, h], CT)
        for k in range(kv):
            nc.sync.dma_start(_w(wv_sb[:, k, :]), _w(wvT[ds(k * P, P), :]))
        wt_sb = consts.tile([P, kt, h], CT)
        for k in range(kt):
            nc.sync.dma_start(_w(wt_sb[:, k, :]), _w(wtT[ds(k * P, P), :]))
        bv_sb = consts.tile([P, mh], F32)
        nc.sync.dma_start(bv_sb[:], bvp[:, :])
        bt_sb = consts.tile([P, mh], F32)
        nc.sync.dma_start(bt_sb[:], btp[:, :])
        ones_sb = consts.tile([P, P], CT)
        nc.sync.dma_start(_w(ones_sb[:]), _w(onesd[:, :]))
        ones_col = ones_sb[:, 0:1]
        ones_row = ones_sb[0:1, :]

        def proj(m_range, kk, w_sb, b_sb, x_sb, y_sb, ysq_sb):
            """y[:, m, :] = W[:, :, m-slice].T @ x + b ; ysq = y*y"""
            for m in m_range:
                for n2 in range(ns):
                    pv = ps_mm.tile([P, fd], F32, tag="ps_mm")
                    for k in range(kk):
                        nc.tensor.matmul(
                            pv[:],
                            _w(w_sb[:, k, ts(m, P)]),
                            _w(x_sb[:, k, ds(n2 * fd, fd)]),
                            start=(k == 0),
                            stop=(k == kk - 1),
                        )
                    nc.scalar.activation(
                        _w(y_sb[:, m, ds(n2 * fd, fd)]), pv[:], AF.Identity,
                        bias=b_sb[:, ds(m, 1)],
                    )
                    nc.vector.tensor_mul(
                        _w(ysq_sb[:, m, ds(n2 * fd, fd)]),
                        y_sb[:, m, ds(n2 * fd, fd)],
                        y_sb[:, m, ds(n2 * fd, fd)],
                    )

        def norm_rows(ysq_sb, tag):
            """Per-column sqrt(sum_h ysq) as ns rows of [1, fd] (dtype CT)."""
            rows = []
            for n2 in range(ns):
                pn = ps_norm.tile([1, fd], F32, tag="ps_norm")
                for m in range(mh):
                    nc.tensor.matmul(
                        pn[:],
                        _w(ones_col),
                        _w(ysq_sb[:, m, ds(n2 * fd, fd)]),
                        start=(m == 0),
                        stop=(m == mh - 1),
                    )
                nrow = row_pool.tile([1, fd], CT, tag=f"n_{tag}{n2}")
                nc.scalar.activation(_w(nrow[:]), pn[:], AF.Sqrt)
                rows.append(nrow)
            return rows

        def repl_recip(rows, dest_sb):
            """Broadcast 1/row across 128 partitions into dest_sb [P, s] f32."""
            for n2 in range(ns):
                rp = ps_repl.tile([P, fd], F32, tag="ps_repl")
                nc.tensor.matmul(rp[:], _w(ones_row), _w(rows[n2][:]))
                nc.vector.reciprocal(dest_sb[:, ds(n2 * fd, fd)], rp[:])

        for b in range(bpc):
            # --- input loads (per-k chunks; Tile orders/overlaps them)
            vis_sb = vis_pool.tile([P, kv, s], CT)
            for k in range(kv):
                nc.sync.dma_start(_w(vis_sb[:, k, :]), _w(visT[b, ds(k * P, P), :]))
            txt_sb = txt_pool.tile([P, kt, s], CT)
            for k in range(kt):
                nc.sync.dma_start(_w(txt_sb[:, k, :]), _w(txtT[b, ds(k * P, P), :]))

            vt_sb = vt_pool.tile([P, mh, s], CT)
            tt_sb = tt_pool.tile([P, mh, s], CT)
            vsq_sb = sq_pool.tile([P, mh, s], CT, tag="vsq")
            tsq_sb = sq_pool.tile([P, mh, s], CT, tag="tsq")

            # --- projections + v-norm chain interleaved to keep PE dense
            proj(range(mh), kv, wv_sb, bv_sb, vis_sb, vt_sb, vsq_sb)
            proj(range(0, mh // 2), kt, wt_sb, bt_sb, txt_sb, tt_sb, tsq_sb)
            rvn_rows = norm_rows(vsq_sb, "v")
            proj(range(mh // 2, mh), kt, wt_sb, bt_sb, txt_sb, tt_sb, tsq_sb)

            # --- replicate 1/vn across partitions, fold into vT
            rvn_bc = rvn_pool.tile([P, s], F32)
            repl_recip(rvn_rows, rvn_bc)
            for m in range(mh):
                for n2 in range(ns):
                    nc.vector.tensor_mul(
                        _w(vt_sb[:, m, ds(n2 * fd, fd)]),
                        vt_sb[:, m, ds(n2 * fd, fd)],
                        rvn_bc[:, ds(n2 * fd, fd)],
                    )

            rtn_rows = norm_rows(tsq_sb, "t")
            rtn_bc = rtn_pool.tile([P, s], F32)

            # --- dots + epilogue
            out_sb = None
            for i in range(ms):
                pds = []
                for jc in range(ns):
                    pd = ps_mm.tile([P, fd], F32, tag="ps_mm")
                    for hc in range(mh):
                        nc.tensor.matmul(
                            pd[:],
                            _w(vt_sb[:, hc, ts(i, P)]),
                            _w(tt_sb[:, hc, ds(jc * fd, fd)]),
                            start=(hc == 0),
                            stop=(hc == mh - 1),
                        )
                    pds.append(pd)
                if i == 0:
                    # 1/tn broadcast, emitted after dots i=0 so PE stays busy
                    # while the t-norm row chain completes
                    repl_recip(rtn_rows, rtn_bc)
                if i % 2 == 0:
                    out_sb = out_pool.tile([P, 2, s], F32)
                for jc in range(ns):
                    nc.vector.tensor_mul(
                        out_sb[:, i % 2, ds(jc * fd, fd)],
                        pds[jc][:],
                        rtn_bc[:, ds(jc * fd, fd)],
                    )
                if i % 2 == 1:
                    nc.sync.dma_start(
                        out[b, ds((i - 1) * P, 2 * P), :].rearrange(
                            "(t p) s -> p t s", p=P
                        ),
                        out_sb[:],
                    )

    nc.compile()
    return nc


_ONES = np.ones((P, P), dtype=np.float32)

_CACHE = {}


def _get_nc(dtype="bf16"):
    if dtype not in _CACHE:
        _CACHE[dtype] = build(dtype=dtype)
    return _CACHE[dtype]


def _prep_in_maps(visual_features, text_features, Wv, bv, Wt, bt, dtype="bf16"):
    import ml_dtypes

    f = np.float32
    ct = ml_dtypes.bfloat16 if dtype == "bf16" else f
    wvT = np.ascontiguousarray(np.asarray(Wv, dtype=f).T).astype(ct)  # [VD, H]
    wtT = np.ascontiguousarray(np.asarray(Wt, dtype=f).T).astype(ct)  # [TD, H]
    bvp = np.ascontiguousarray(np.asarray(bv, dtype=f).reshape(H // P, P).T)
    btp = np.ascontiguousarray(np.asarray(bt, dtype=f).reshape(H // P, P).T)
    ones = _ONES.astype(ct)
    vis = np.asarray(visual_features, dtype=f)
    txt = np.asarray(text_features, dtype=f)
    in_maps = []
    for c in range(NCORES):
        sl = slice(c * BPC, (c + 1) * BPC)
        in_maps.append({
            "visT": np.ascontiguousarray(vis[sl].transpose(0, 2, 1)).astype(ct),
            "txtT": np.ascontiguousarray(txt[sl].transpose(0, 2, 1)).astype(ct),
            "wvT": wvT,
            "wtT": wtT,
            "bvp": bvp,
            "btp": btp,
            "ones": ones,
        })
    return in_maps


def run(inputs, trace=False, tmpdir=None, dtype="bf16"):
    """Returns (full_output, BassKernelResults)."""
    from concourse.bass_utils import run_bass_kernel_spmd

    nc = _get_nc(dtype)
    in_maps = _prep_in_maps(**inputs, dtype=dtype)
    res = run_bass_kernel_spmd(
        nc, in_maps, core_ids=list(range(NCORES)), trace=trace, tmpdir=tmpdir
    )
    outp = np.concatenate([res.results[c]["out"] for c in range(NCORES)], axis=0)
    return outp, res


def kernel(**inputs) -> np.ndarray:
    outp, _ = run(inputs, trace=False)
    return outp


# revision 4
# speedup vs baseline: 1.2723x; 1.2547x over previous
"""CrossModalMatchingNetwork Trainium2 kernel.

Full-input contract: kernel(**inputs) takes the unsharded numpy inputs and
returns the full [B, S, S] cosine-similarity output (float32).

Strategy: data-parallel over batch across 8 NeuronCores (2 batches/core).
Host-side prep transposes the big activations to [D, S] layout so the
contraction dim lands on SBUF partitions, casts them to bf16 (fp32 PSUM
accumulation keeps the error ~3e-3), and replicates the small projection
weights (pre-transposed to [D, H]) to every core.

Per core, per batch:
  vT[h,s]  = sum_d WvT[d,h] * visT[d,s] + bv[h]     (bf16 matmuls, fp32 acc)
  tT[h,s]  = sum_d WtT[d,h] * txtT[d,s] + bt[h]
  vn2[s]   = sum_h vT[h,s]^2   (DVE squares + ones-vector matmul)
  tn2[s]   = sum_h tT[h,s]^2
  vT      *= 1/sqrt(vn2)       (sqrt row -> replicate via K=1 matmul -> DVE
                                reciprocal on all 128 lanes -> fold into vT)
  dots     = vT.T @ tT
  out      = dots * 1/sqrt(tn2)  (DVE epilogue, fp32 out)
"""

import numpy as np
from contextlib import ExitStack

import concourse.bass as bass
import concourse.mybir as mybir
import concourse.tile as tile
from concourse import bacc
from concourse.bass import ds, ts

B, S, VD, TD, H = 16, 1024, 1024, 768, 512
NCORES = 8
BPC = B // NCORES  # batches per core
P = 128
FD = 512  # matmul moving-operand free dim (one PSUM bank of fp32)

F32 = mybir.dt.float32
F32R = mybir.dt.float32r
BF16 = mybir.dt.bfloat16

AF = mybir.ActivationFunctionType


def build(bpc=BPC, s=S, vd=VD, td=TD, h=H, dtype="bf16"):
    fd = min(FD, s)
    kv, kt, mh = vd // P, td // P, h // P
    ns, ms = s // fd, s // P

    if dtype == "bf16":
        CT = BF16
        _w = lambda ap: ap  # noqa: E731
    else:
        CT = F32
        _w = lambda ap: ap.bitcast(F32R)  # noqa: E731  # fp32r: single-pass PE

    nc = bacc.Bacc("TRN2", target_bir_lowering=False)
    visT = nc.dram_tensor("visT", [bpc, vd, s], CT, kind="ExternalInput")
    txtT = nc.dram_tensor("txtT", [bpc, td, s], CT, kind="ExternalInput")
    wvT = nc.dram_tensor("wvT", [vd, h], CT, kind="ExternalInput")
    wtT = nc.dram_tensor("wtT", [td, h], CT, kind="ExternalInput")
    bvp = nc.dram_tensor("bvp", [P, mh], F32, kind="ExternalInput")
    btp = nc.dram_tensor("btp", [P, mh], F32, kind="ExternalInput")
    onesd = nc.dram_tensor("ones", [P, P], CT, kind="ExternalInput")
    out = nc.dram_tensor("out", [bpc, s, s], F32, kind="ExternalOutput")

    with (
        tile.TileContext(nc) as tc,
        ExitStack() as ctx,
        nc.allow_low_precision(reason="compute dtype is bf16/fp32r by design"),
    ):
        consts = ctx.enter_context(tc.tile_pool(name="consts", bufs=1))
        vis_pool = ctx.enter_context(tc.tile_pool(name="vis", bufs=1))
        txt_pool = ctx.enter_context(tc.tile_pool(name="txt", bufs=1))
        vt_pool = ctx.enter_context(tc.tile_pool(name="vt", bufs=1))
        tt_pool = ctx.enter_context(tc.tile_pool(name="tt", bufs=1))
        sq_pool = ctx.enter_context(tc.tile_pool(name="sq", bufs=1))
        row_pool = ctx.enter_context(tc.tile_pool(name="rows", bufs=2))
        rvn_pool = ctx.enter_context(tc.tile_pool(name="rvn", bufs=2))
        rtn_pool = ctx.enter_context(tc.tile_pool(name="rtn", bufs=2))
        out_pool = ctx.enter_context(tc.tile_pool(name="outs", bufs=2))
        ps_mm = ctx.enter_context(tc.tile_pool(name="ps_mm", bufs=3, space="PSUM"))
        ps_repl = ctx.enter_context(tc.tile_pool(name="ps_repl", bufs=2, space="PSUM"))
        ps_norm = ctx.enter_context(tc.tile_pool(name="ps_norm", bufs=2, space="PSUM"))

        # --- constants: weights (as [P, k, h]), partition-major biases, ones
        wv_sb = consts.tile([P, kv, h], CT)
        for k in range(kv):
            nc.sync.dma_start(_w(wv_sb[:, k, :]), _w(wvT[ds(k * P, P), :]))
        wt_sb = consts.tile([P, kt, h], CT)
        for k in range(kt):
            nc.sync.dma_start(_w(wt_sb[:, k, :]), _w(wtT[ds(k * P, P), :]))
        bv_sb = consts.tile([P, mh], F32)
        nc.sync.dma_start(bv_sb[:], bvp[:, :])
        bt_sb = consts.tile([P, mh], F32)
        nc.sync.dma_start(bt_sb[:], btp[:, :])
        ones_sb = consts.tile([P, P], CT)
        nc.sync.dma_start(_w(ones_sb[:]), _w(onesd[:, :]))
        ones_col = ones_sb[:, 0:1]
        ones_row = ones_sb[0:1, :]

        def proj(m_range, kk, w_sb, b_sb, x_sb, y_sb, ysq_sb):
            """y[:, m, :] = W[:, :, m-slice].T @ x + b ; ysq = y*y"""
            for m in m_range:
                for n2 in range(ns):
                    pv = ps_mm.tile([P, fd], F32, tag="ps_mm")
                    for k in range(kk):
                        nc.tensor.matmul(
                            pv[:],
                            _w(w_sb[:, k, ts(m, P)]),
                            _w(x_sb[:, k, ds(n2 * fd, fd)]),
                            start=(k == 0),
                            stop=(k == kk - 1),
                        )
                    nc.scalar.activation(
                        _w(y_sb[:, m, ds(n2 * fd, fd)]), pv[:], AF.Identity,
                        bias=b_sb[:, ds(m, 1)],
                    )
                    nc.vector.tensor_mul(
                        _w(ysq_sb[:, m, ds(n2 * fd, fd)]),
                        y_sb[:, m, ds(n2 * fd, fd)],
                        y_sb[:, m, ds(n2 * fd, fd)],
                    )

        def norm_rows(ysq_sb, tag):
            """Per-column sqrt(sum_h ysq) as ns rows of [1, fd] (dtype CT)."""
            rows = []
            for n2 in range(ns):
                pn = ps_norm.tile([1, fd], F32, tag="ps_norm")
                for m in range(mh):
                    nc.tensor.matmul(
                        pn[:],
                        _w(ones_col),
                        _w(ysq_sb[:, m, ds(n2 * fd, fd)]),
                        start=(m == 0),
                        stop=(m == mh - 1),
                    )
                nrow = row_pool.tile([1, fd], CT, tag=f"n_{tag}{n2}")
                nc.scalar.activation(_w(nrow[:]), pn[:], AF.Sqrt)
                rows.append(nrow)
            return rows

        def repl_recip(rows, dest_sb):
            """Broadcast 1/row across 128 partitions into dest_sb [P, s] f32."""
            for n2 in range(ns):
                rp = ps_repl.tile([P, fd], F32, tag="ps_repl")
                nc.tensor.matmul(rp[:], _w(ones_row), _w(rows[n2][:]))
                nc.vector.reciprocal_approx_fast(
                    out=dest_sb[:, ds(n2 * fd, fd)], in_=rp[:]
                )

        for b in range(bpc):
            # --- input loads (per-k chunks; Tile orders/overlaps them)
            vis_sb = vis_pool.tile([P, kv, s], CT)
            for k in range(kv):
                nc.sync.dma_start(_w(vis_sb[:, k, :]), _w(visT[b, ds(k * P, P), :]))
            txt_sb = txt_pool.tile([P, kt, s], CT)
            for k in range(kt):
                nc.sync.dma_start(_w(txt_sb[:, k, :]), _w(txtT[b, ds(k * P, P), :]))

            vt_sb = vt_pool.tile([P, mh, s], CT)
            tt_sb = tt_pool.tile([P, mh, s], CT)
            vsq_sb = sq_pool.tile([P, mh, s], CT, tag="vsq")
            tsq_sb = sq_pool.tile([P, mh, s], CT, tag="tsq")

            # --- projections + v-norm chain interleaved to keep PE dense
            proj(range(mh), kv, wv_sb, bv_sb, vis_sb, vt_sb, vsq_sb)
            proj(range(0, mh // 2), kt, wt_sb, bt_sb, txt_sb, tt_sb, tsq_sb)
            rvn_rows = norm_rows(vsq_sb, "v")
            # replicate 1/vn and fold into vT; the chain overlaps proj-t m23
            rvn_bc = rvn_pool.tile([P, s], F32)
            repl_recip(rvn_rows, rvn_bc)
            proj(range(mh // 2, mh), kt, wt_sb, bt_sb, txt_sb, tt_sb, tsq_sb)
            for n2 in range(ns):
                for m in range(mh):
                    nc.vector.tensor_mul(
                        _w(vt_sb[:, m, ds(n2 * fd, fd)]),
                        vt_sb[:, m, ds(n2 * fd, fd)],
                        rvn_bc[:, ds(n2 * fd, fd)],
                    )

            rtn_rows = norm_rows(tsq_sb, "t")
            rtn_bc = rtn_pool.tile([P, s], F32)
            repl_recip(rtn_rows, rtn_bc)

            # --- dots + epilogue
            out_sb = None
            for i in range(ms):
                pds = []
                for jc in range(ns):
                    pd = ps_mm.tile([P, fd], F32, tag="ps_mm")
                    for hc in range(mh):
                        nc.tensor.matmul(
                            pd[:],
                            _w(vt_sb[:, hc, ts(i, P)]),
                            _w(tt_sb[:, hc, ds(jc * fd, fd)]),
                            start=(hc == 0),
                            stop=(hc == mh - 1),
                        )
                    pds.append(pd)
                if i % 2 == 0:
                    out_sb = out_pool.tile([P, 2, s], F32)
                for jc in range(ns):
                    nc.vector.tensor_mul(
                        out_sb[:, i % 2, ds(jc * fd, fd)],
                        pds[jc][:],
                        rtn_bc[:, ds(jc * fd, fd)],
                    )
                if i % 2 == 1:
                    nc.sync.dma_start(
                        out[b, ds((i - 1) * P, 2 * P), :].rearrange(
                            "(t p) s -> p t s", p=P
                        ),
                        out_sb[:],
                    )

    nc.compile()
    return nc


_ONES = np.ones((P, P), dtype=np.float32)

_CACHE = {}


def _get_nc(dtype="bf16"):
    if dtype not in _CACHE:
        _CACHE[dtype] = build(dtype=dtype)
    return _CACHE[dtype]


def _prep_in_maps(visual_features, text_features, Wv, bv, Wt, bt, dtype="bf16"):
    import ml_dtypes

    f = np.float32
    ct = ml_dtypes.bfloat16 if dtype == "bf16" else f
    wvT = np.ascontiguousarray(np.asarray(Wv, dtype=f).T).astype(ct)  # [VD, H]
    wtT = np.ascontiguousarray(np.asarray(Wt, dtype=f).T).astype(ct)  # [TD, H]
    bvp = np.ascontiguousarray(np.asarray(bv, dtype=f).reshape(H // P, P).T)
    btp = np.ascontiguousarray(np.asarray(bt, dtype=f).reshape(H // P, P).T)
    ones = _ONES.astype(ct)
    vis = np.asarray(visual_features, dtype=f)
    txt = np.asarray(text_features, dtype=f)
    in_maps = []
    for c in range(NCORES):
        sl = slice(c * BPC, (c + 1) * BPC)
        in_maps.append({
            "visT": np.ascontiguousarray(vis[sl].transpose(0, 2, 1)).astype(ct),
            "txtT": np.ascontiguousarray(txt[sl].transpose(0, 2, 1)).astype(ct),
            "wvT": wvT,
            "wtT": wtT,
            "bvp": bvp,
            "btp": btp,
            "ones": ones,
        })
    return in_maps


def run(inputs, trace=False, tmpdir=None, dtype="bf16"):
    """Returns (full_output, BassKernelResults)."""
    from concourse.bass_utils import run_bass_kernel_spmd

    nc = _get_nc(dtype)
    in_maps = _prep_in_maps(**inputs, dtype=dtype)
    res = run_bass_kernel_spmd(
        nc, in_maps, core_ids=list(range(NCORES)), trace=trace, tmpdir=tmpdir
    )
    outp = np.concatenate([res.results[c]["out"] for c in range(NCORES)], axis=0)
    return outp, res


def kernel(**inputs) -> np.ndarray:
    outp, _ = run(inputs, trace=False)
    return outp
